# revision 1
# baseline (speedup 1.0000x reference)
"""Multi-head self-attention TRN2 kernel (data-parallel over batch).

Problem: B=8, S=1024, D=384, H=8, per-head full D->D projections,
causal + key-padding mask, softmax, out_linear (H*D)->D, query-mask output.

Sharding: batch b -> NeuronCore b (8 cores, no collectives).

Per-core dataflow (one batch element), transpose-free "T-native" layout:
  xT [D,S] resident in SBUF.
  For each head h:
    QT[e,s], KT[e,t] via (lhsT=W chunk, rhs=xT) matmuls (+bias)
    V[t,e] natural via (lhsT=xT chunk, rhs=Wv)   (+bias bcast)
    per q-tile group (4 q-tiles = 512 s-columns), causally-live t-chunks only:
      scoresT[t,s] psum = KT-stationary @ QT       (raw, unscaled)
      masked = min(scoresT, MT[t-chunk])           (DVE, in-psum)
      attnT[t,s] = exp(masked * inv_sqrt_d)        (ACT, direct to f32r SBUF)
      colsums[1,s] += ones^T @ attnT               (PE, M=1 matmul)
    colsums -> DRAM bounce -> [128,4] per-qt columns -> reciprocal
    headT[e,s] = V-stationary @ attnT              (PE)
    out_acc[s,:] += (headT^T @ Wo_h) * recip[s]    (PE + one DVE STT)
  out[s,:] = out_acc * maskq[s]  -> DRAM

No row-max subtraction: masked fill NEG scales to exactly -87, so exp args
stay in [-87, ~25], safe in fp32. All-invalid rows would be wrong (uniform
over a partial window) but those rows always have maskq[s]=0 and are zeroed.
Masking uses min(scores, MT) where MT[t,s] = +BIG if (t<=s and mask[t])
else NEG, matching the reference's where(valid, scores, -1e4) semantics.
"""

import os
from contextlib import ExitStack

import numpy as np

B, S, D, H = 8, 1024, 384, 8
P = 128
DC = D // P          # 3 partition chunks of the d/e axes
NQT = S // P         # 8 q/t tiles of 128
G = 4                # q-tiles per group (s-block = 512)
NG = NQT // G
BIG = 3.0e38
INV_SQRT_D = float(1.0 / np.sqrt(np.float32(D), dtype=np.float32))
NEG = float(-87.0 / INV_SQRT_D)  # raw-score fill; scaled -> -87

# matmul dtype knobs per stage: "f32" | "f32r" | "bf16"
CFG = {
    "proj": os.environ.get("MHA_DT_PROJ", "f32r"),
    "qk": os.environ.get("MHA_DT_QK", "f32r"),
    "pv": os.environ.get("MHA_DT_PV", "f32r"),
    "op": os.environ.get("MHA_DT_OP", "f32r"),
}

_BUILT = None  # (nc, cfg)


def _dt(kind):
    import concourse.mybir as mybir

    if kind == "bf16":
        return mybir.dt.bfloat16
    if kind == "f32r":
        return mybir.dt.float32r
    return mybir.dt.float32


def _np_dt(kind):
    import ml_dtypes

    return ml_dtypes.bfloat16 if kind == "bf16" else np.float32


def build(cfg=None):
    import concourse.bass as bass
    import concourse.bacc as bacc
    import concourse.tile as tile
    import concourse.mybir as mybir

    cfg = dict(CFG if cfg is None else cfg)
    f32 = mybir.dt.float32
    u32 = mybir.dt.uint32
    dt_proj = _dt(cfg["proj"])   # xT, Wq, Wk, Wv tiles
    dt_qk = _dt(cfg["qk"])       # QT, KT tiles
    dt_pv = _dt(cfg["pv"])       # attnT, V, ones tiles
    dt_op = _dt(cfg["op"])       # headT, Wo tiles

    nc = bacc.Bacc("TRN2", target_bir_lowering=False, debug=False)

    xT_d = nc.dram_tensor("xT", [D, S], dt_proj, kind="ExternalInput")
    wq_d = nc.dram_tensor("Wq", [H, D, D], dt_proj, kind="ExternalInput")
    wk_d = nc.dram_tensor("Wk", [H, D, D], dt_proj, kind="ExternalInput")
    wv_d = nc.dram_tensor("Wv", [H, D, D], dt_proj, kind="ExternalInput")
    wo_d = nc.dram_tensor("Wo", [H * D, D], dt_op, kind="ExternalInput")
    bq_d = nc.dram_tensor("bq", [H, D], f32, kind="ExternalInput")
    bk_d = nc.dram_tensor("bk", [H, D], f32, kind="ExternalInput")
    bv_d = nc.dram_tensor("bv", [H, P, D], f32, kind="ExternalInput")
    bo_d = nc.dram_tensor("bo", [P, D], f32, kind="ExternalInput")
    kbigT_d = nc.dram_tensor("kbigT", [P, NQT], f32, kind="ExternalInput")
    maskq_d = nc.dram_tensor("maskq", [S], f32, kind="ExternalInput")
    out_d = nc.dram_tensor("out", [S, D], f32, kind="ExternalOutput")
    # per-(head, group) bounce rows for column sums
    scr_d = nc.dram_tensor("sum_scratch", [H * NG, 512], f32)

    with tile.TileContext(nc) as tc, ExitStack() as ctx:
        consts = ctx.enter_context(tc.tile_pool(name="consts", bufs=1))
        wpool = ctx.enter_context(tc.tile_pool(name="wpool", bufs=2))
        qkv = ctx.enter_context(tc.tile_pool(name="qkv", bufs=1))
        tpool = ctx.enter_context(tc.tile_pool(name="tpool", bufs=1))
        hpool = ctx.enter_context(tc.tile_pool(name="hpool", bufs=2))
        small = ctx.enter_context(tc.tile_pool(name="small", bufs=8))
        opool = ctx.enter_context(tc.tile_pool(name="opool", bufs=2))
        ps_a = ctx.enter_context(tc.tile_pool(name="ps_a", bufs=4, space="PSUM"))
        ps_sm = ctx.enter_context(tc.tile_pool(name="ps_sm", bufs=2, space="PSUM"))
        ps_u = ctx.enter_context(tc.tile_pool(name="ps_u", bufs=2, space="PSUM"))

        # ---- PE warm-up: keep the array busy under the initial DMA shadow
        # so the HAM clock-gate is released (2.4 GHz) before real matmuls.
        warm = consts.tile([P, P], dt_proj, tag="warm")
        wz = warm.bitcast(u32) if dt_proj == mybir.dt.float32r else warm
        nc.vector.memset(wz, 0)
        ps_w = ps_sm.tile([P, 512], f32, tag="sm", name="ps_warm")
        for _ in range(24):
            nc.tensor.matmul(ps_w[:, :P], warm, warm, start=True, stop=True)

        # ---- setup ----
        xT_sb = consts.tile([P, DC, S], dt_proj, tag="xT")
        nc.sync.dma_start(out=xT_sb, in_=xT_d.ap().rearrange("(c p) s -> p c s", p=P))

        kbigT_sb = consts.tile([P, NQT], f32, tag="kbigT")
        nc.sync.dma_start(out=kbigT_sb, in_=kbigT_d.ap())

        maskq_sb = consts.tile([P, NQT], f32, tag="maskq")
        nc.sync.dma_start(
            out=maskq_sb, in_=maskq_d.ap().rearrange("(q p) -> p q", p=P)
        )

        bo_sb = consts.tile([P, D], f32, tag="bo")
        nc.sync.dma_start(out=bo_sb, in_=bo_d.ap())

        ones_sb = consts.tile([P, 1], dt_pv, tag="ones")
        if dt_pv == mybir.dt.float32r:
            nc.vector.memset(ones_sb.bitcast(u32), 0x3F800000)
        else:
            nc.vector.memset(ones_sb, 1.0)

        # MT[t, s] = kbig[t] where s >= t else NEG   (per 128-chunk of t)
        msk = consts.tile([P, NQT, S], f32, tag="M")
        for tt in range(NQT):
            nc.vector.memset(msk[:, tt, :], 0.0)
            nc.vector.tensor_scalar_add(
                out=msk[:, tt, :], in0=msk[:, tt, :],
                scalar1=kbigT_sb[:, tt : tt + 1],
            )
            nc.gpsimd.affine_select(
                out=msk[:, tt, :],
                in_=msk[:, tt, :],
                compare_op=mybir.AluOpType.is_ge,
                fill=NEG,
                base=-tt * P,
                channel_multiplier=-1,
                pattern=[[1, S]],
            )

        out_acc = consts.tile([P, NQT, D], f32, tag="out_acc")
        for qt in range(NQT):
            nc.vector.tensor_copy(out=out_acc[:, qt, :], in_=bo_sb)

        # ---- per-head pipeline ----
        n_heads = int(os.environ.get("MHA_HEADS", str(H)))
        for h in range(n_heads):
            wq_sb = wpool.tile([P, DC, D], dt_proj, tag="wq")
            wk_sb = wpool.tile([P, DC, D], dt_proj, tag="wk")
            wv_sb = wpool.tile([P, DC, D], dt_proj, tag="wv")
            wo_sb = wpool.tile([P, DC, D], dt_op, tag="wo")
            nc.sync.dma_start(
                out=wq_sb, in_=wq_d.ap()[h].rearrange("(c p) e -> p c e", p=P)
            )
            nc.sync.dma_start(
                out=wk_sb, in_=wk_d.ap()[h].rearrange("(c p) e -> p c e", p=P)
            )
            nc.sync.dma_start(
                out=wv_sb, in_=wv_d.ap()[h].rearrange("(c p) e -> p c e", p=P)
            )
            nc.sync.dma_start(
                out=wo_sb,
                in_=wo_d.ap()[h * D : (h + 1) * D, :].rearrange(
                    "(c p) e -> p c e", p=P
                ),
            )
            bq_sb = wpool.tile([P, DC], f32, tag="bq")
            bk_sb = wpool.tile([P, DC], f32, tag="bk")
            nc.sync.dma_start(out=bq_sb, in_=bq_d.ap()[h].rearrange("(c p) -> p c", p=P))
            nc.sync.dma_start(out=bk_sb, in_=bk_d.ap()[h].rearrange("(c p) -> p c", p=P))
            bv_sb = wpool.tile([P, D], f32, tag="bv")
            nc.sync.dma_start(out=bv_sb, in_=bv_d.ap()[h])

            # QT/KT [e, s] projections (psum 512-wide per (ec, sh))
            qt_sb = qkv.tile([P, DC, S], dt_qk, tag="QT")
            kt_sb = qkv.tile([P, DC, S], dt_qk, tag="KT")
            for dst, w_sb, b_sb in ((kt_sb, wk_sb, bk_sb), (qt_sb, wq_sb, bq_sb)):
                for ec in range(DC):
                    for sh in range(S // 512):
                        ps = ps_a.tile([P, 512], f32, tag="a")
                        for dc in range(DC):
                            nc.tensor.matmul(
                                ps,
                                w_sb[:, dc, ec * P : (ec + 1) * P],
                                xT_sb[:, dc, sh * 512 : (sh + 1) * 512],
                                start=(dc == 0),
                                stop=(dc == DC - 1),
                            )
                        nc.scalar.activation(
                            out=dst[:, ec, sh * 512 : (sh + 1) * 512],
                            in_=ps,
                            func=mybir.ActivationFunctionType.Identity,
                            bias=b_sb[:, ec : ec + 1],
                        )

            # V [t, e] natural
            v_sb = qkv.tile([P, NQT, D], dt_pv, tag="V")
            for tt in range(NQT):
                psv = ps_sm.tile([P, 512], f32, tag="sm")
                for dc in range(DC):
                    nc.tensor.matmul(
                        psv[:, :D],
                        xT_sb[:, dc, tt * P : (tt + 1) * P],
                        wv_sb[:, dc, :],
                        start=(dc == 0),
                        stop=(dc == DC - 1),
                    )
                nc.vector.tensor_add(out=v_sb[:, tt, :], in0=psv[:, :D], in1=bv_sb)

            # attention per 512-wide s-group, causally-live t-chunks only
            for qg in range(NG):
                ntt = qg * G + G  # live t-chunks for this group
                s0 = qg * 512
                att_t = tpool.tile([P, NQT, 512], dt_pv, tag="attnT", name="att_t")
                ps_sums = ps_u.tile([1, 512], f32, tag="u")
                for tt in range(ntt):
                    ps_sc = ps_a.tile([P, 512], f32, tag="a")
                    for ec in range(DC):
                        nc.tensor.matmul(
                            ps_sc,
                            kt_sb[:, ec, tt * P : (tt + 1) * P],
                            qt_sb[:, ec, s0 : s0 + 512],
                            start=(ec == 0),
                            stop=(ec == DC - 1),
                        )
                    nc.vector.tensor_tensor(
                        out=ps_sc,
                        in0=ps_sc,
                        in1=msk[:, tt, s0 : s0 + 512],
                        op=mybir.AluOpType.min,
                    )
                    nc.scalar.activation(
                        out=att_t[:, tt, :],
                        in_=ps_sc,
                        func=mybir.ActivationFunctionType.Exp,
                        scale=INV_SQRT_D,
                    )
                for tt in range(ntt):
                    nc.tensor.matmul(
                        ps_sums,
                        ones_sb,
                        att_t[:, tt, :],
                        start=(tt == 0),
                        stop=(tt == ntt - 1),
                    )
                # column sums -> DRAM bounce -> per-qt [128,1] recips
                srow = small.tile([1, 512], f32, tag="srow")
                nc.vector.tensor_copy(out=srow, in_=ps_sums)
                scr = scr_d.ap()[h * NG + qg]
                nc.sync.dma_start(out=scr, in_=srow)
                scat = small.tile([P, G], f32, tag="scat")
                nc.sync.dma_start(
                    out=scat,
                    in_=bass.AP(tensor=scr.tensor, offset=scr.offset, ap=[[1, P], [P, G]]),
                )
                recip = small.tile([P, G], f32, tag="recip")
                nc.vector.reciprocal(out=recip, in_=scat)

                # headT [e, s-group] = sum_t V-stationary @ attnT
                head_sb = hpool.tile([P, DC, 512], dt_op, tag="headT")
                for ec in range(DC):
                    pspv = ps_sm.tile([P, 512], f32, tag="sm")
                    for tt in range(ntt):
                        nc.tensor.matmul(
                            pspv,
                            v_sb[:, tt, ec * P : (ec + 1) * P],
                            att_t[:, tt, :],
                            start=(tt == 0),
                            stop=(tt == ntt - 1),
                        )
                    nc.scalar.copy(out=head_sb[:, ec, :], in_=pspv)

                # out projection for this head, accumulate with 1/colsum
                for qi in range(G):
                    qt = qg * G + qi
                    psop = ps_sm.tile([P, 512], f32, tag="sm")
                    for ec in range(DC):
                        nc.tensor.matmul(
                            psop[:, :D],
                            head_sb[:, ec, qi * P : (qi + 1) * P],
                            wo_sb[:, ec, :],
                            start=(ec == 0),
                            stop=(ec == DC - 1),
                        )
                    nc.vector.scalar_tensor_tensor(
                        out=out_acc[:, qt, :],
                        in0=psop[:, :D],
                        scalar=recip[:, qi : qi + 1],
                        in1=out_acc[:, qt, :],
                        op0=mybir.AluOpType.mult,
                        op1=mybir.AluOpType.add,
                    )

        # ---- final query-mask + store ----
        for qt in range(NQT):
            st = opool.tile([P, D], f32, tag="store")
            nc.vector.tensor_scalar_mul(
                out=st, in0=out_acc[:, qt, :], scalar1=maskq_sb[:, qt : qt + 1]
            )
            nc.sync.dma_start(out=out_d.ap()[qt * P : (qt + 1) * P, :], in_=st)

    nc.compile()
    return nc


def _in_maps(x, mask, Wq, bq, Wk, bk, Wv, bv, Wo, bo, cfg):
    np_proj = _np_dt(cfg["proj"])
    np_op = _np_dt(cfg["op"])
    x = np.asarray(x, np.float32)
    xT = np.ascontiguousarray(x.transpose(0, 2, 1))  # [B, D, S]
    m = np.asarray(mask) != 0
    kbig = np.where(m, np.float32(BIG), np.float32(NEG)).astype(np.float32)
    maskq = m.astype(np.float32)
    shared = {
        "Wq": np.asarray(Wq, np_proj),
        "Wk": np.asarray(Wk, np_proj),
        "Wv": np.asarray(Wv, np_proj),
        "Wo": np.asarray(Wo, np_op),
        "bq": np.asarray(bq, np.float32),
        "bk": np.asarray(bk, np.float32),
        "bv": np.broadcast_to(
            np.asarray(bv, np.float32)[:, None, :], (H, P, D)
        ).copy(),
        "bo": np.broadcast_to(np.asarray(bo, np.float32)[None, :], (P, D)).copy(),
    }
    return [
        {
            "xT": xT[b].astype(np_proj),
            "kbigT": np.ascontiguousarray(kbig[b].reshape(NQT, P).T),
            "maskq": maskq[b],
            **shared,
        }
        for b in range(B)
    ]


def run(inputs, trace=False, cfg=None):
    """inputs: dict from setup_inputs(). Returns (out [B,S,D] f32, results)."""
    from concourse.bass_utils import run_bass_kernel_spmd

    global _BUILT
    cfg = dict(CFG if cfg is None else cfg)
    if _BUILT is None or _BUILT[1] != cfg:
        _BUILT = (build(cfg), cfg)
    nc = _BUILT[0]
    in_maps = _in_maps(**inputs, cfg=cfg)
    res = run_bass_kernel_spmd(
        nc, in_maps, core_ids=list(range(B)), trace=trace
    )
    out = np.stack([np.asarray(res.results[b]["out"], np.float32) for b in range(B)])
    return out, res


def kernel(**inputs):
    out, _ = run(inputs, trace=False)
    return out



# revision 6
# speedup vs baseline: 1.3769x; 1.3769x over previous
"""Multi-head self-attention TRN2 kernel (data-parallel over batch).

Problem: B=8, S=1024, D=384, H=8, per-head full D->D projections,
causal + key-padding mask, softmax, out_linear (H*D)->D, query-mask output.

Sharding: batch b -> NeuronCore b (8 cores, no collectives).

Algebraic restructure (host precompute, exact):
  M_h = Wq_h @ Wk_h^T   ->  scores_raw = x M_h x^T   (K-projection eliminated)
  N_h = Wv_h @ Wo_h     ->  out = sum_h softmax(scores) @ (x N_h)  (out-proj eliminated)
  bias folds: Q.bk term is constant per query row -> cancels in softmax;
  bq.K term -> per-key exp bias column; (attn@bv)Wo = bv@Wo (softmax sums to 1)
  -> folded into bo on host. All biases are zero in this problem anyway.

Per-core dataflow (one batch element), transpose-free, all bf16 matmuls:
  xT [D,S] bf16 resident in SBUF (used 3 ways: P1/P2 moving, P3 stationary).
  For each head h:
    P1: Q'T[e,s] = M-chunks @ xT          (PE, psum [128,512], -> bf16 SBUF)
    P2: U[t,e]   = xT-chunks @ N          (PE, psum [128,384], -> bf16 SBUF)
    per 256-wide q group (4 groups, causally-live t-chunks only):
      P3: scoresT[t,s] psum = xT-chunk stationary @ Q'T   (raw, unscaled)
      diagonal chunks: min(scores, MTpat) in-psum (DVE), 2 const patterns
      attnT[t,s] = exp(scores*inv_sqrt_d + keybias[t]) -> bf16 SBUF (ACT)
      colsum[1,s] += ones^T @ attnT       (PE, M=1 matmul, per 512-half)
    per 512-half: colsum -> DRAM bounce -> [128,4] -> +eps -> recip -> *maskq
    P4 per q-tile (128): psum[s,e] = sum_t attnT-chunk stationary @ U
      out_acc[s,:] += psum * recip'[s]    (DVE STT)
  out = out_acc (maskq,bo pre-folded) -> DRAM
"""

import os
from contextlib import ExitStack

import numpy as np

B, S, D, H = 8, 1024, 384, 8
P = 128
DC = D // P          # 3 partition chunks of the d/e axes
NQT = S // P         # 8 q/t tiles of 128
GW = 256             # q-group width for scores/exp
NG = S // GW         # 4 groups
BIG = 3.0e38
INV_SQRT_D = float(1.0 / np.sqrt(np.float32(D), dtype=np.float32))
KNEG = -120.0                      # exp bias for masked keys -> exp==0 in bf16
RAWNEG = float(KNEG / INV_SQRT_D)  # raw-score causal fill; scaled -> -120

CFG = {"dt": os.environ.get("MHA_DT", "bf16")}

_BUILT = None  # (nc, cfg)


def _dt(kind):
    import concourse.mybir as mybir

    return {"bf16": mybir.dt.bfloat16, "f32r": mybir.dt.float32r,
            "f32": mybir.dt.float32}[kind]


def _np_dt(kind):
    import ml_dtypes

    return ml_dtypes.bfloat16 if kind == "bf16" else np.float32


def build(cfg=None):
    import concourse.bass as bass
    import concourse.bacc as bacc
    import concourse.tile as tile
    import concourse.mybir as mybir

    cfg = dict(CFG if cfg is None else cfg)
    f32 = mybir.dt.float32
    u32 = mybir.dt.uint32
    dt = _dt(cfg["dt"])

    nc = bacc.Bacc("TRN2", target_bir_lowering=False, debug=False)

    xT_d = nc.dram_tensor("xT", [D, S], dt, kind="ExternalInput")
    m_d = nc.dram_tensor("M", [H, D, D], dt, kind="ExternalInput")
    n_d = nc.dram_tensor("N", [H, D, D], dt, kind="ExternalInput")
    kb_d = nc.dram_tensor("kbT", [P, NQT], f32, kind="ExternalInput")
    maskq_d = nc.dram_tensor("maskq", [S], f32, kind="ExternalInput")
    bo_d = nc.dram_tensor("bo", [P, D], f32, kind="ExternalInput")
    out_d = nc.dram_tensor("out", [S, D], f32, kind="ExternalOutput")
    # per-(head, s-half) bounce rows for column sums
    scr_d = nc.dram_tensor("sum_scratch", [H * 2, 512], f32)

    with tile.TileContext(nc) as tc, ExitStack() as ctx:
        consts = ctx.enter_context(tc.tile_pool(name="consts", bufs=1))
        wpool = ctx.enter_context(tc.tile_pool(name="wpool", bufs=2))
        qpool = ctx.enter_context(tc.tile_pool(name="qpool", bufs=2))
        upool = ctx.enter_context(tc.tile_pool(name="upool", bufs=2))
        apool = ctx.enter_context(tc.tile_pool(name="apool", bufs=3))
        small = ctx.enter_context(tc.tile_pool(name="small", bufs=8))
        ps_pj = ctx.enter_context(tc.tile_pool(name="ps_pj", bufs=2, space="PSUM"))
        ps_sc = ctx.enter_context(tc.tile_pool(name="ps_sc", bufs=2, space="PSUM"))
        ps_pv = ctx.enter_context(tc.tile_pool(name="ps_pv", bufs=2, space="PSUM"))
        ps_cs = ctx.enter_context(tc.tile_pool(name="ps_cs", bufs=1, space="PSUM"))

        # ---- PE warm-up under the initial DMA shadow (release clock gate)
        warm = consts.tile([P, P], dt, tag="warm")
        nc.vector.memset(warm, 0)
        ps_w = ps_pj.tile([P, 512], f32, tag="pj", name="ps_warm")
        for _ in range(24):
            nc.tensor.matmul(ps_w[:, :P], warm, warm, start=True, stop=True)

        # ---- setup ----
        xT_sb = consts.tile([P, DC, S], dt, tag="xT")
        nc.sync.dma_start(out=xT_sb, in_=xT_d.ap().rearrange("(c p) s -> p c s", p=P))

        kb_sb = consts.tile([P, NQT], f32, tag="kbT")
        nc.sync.dma_start(out=kb_sb, in_=kb_d.ap())

        maskq_sb = consts.tile([P, NQT], f32, tag="maskq")
        nc.sync.dma_start(
            out=maskq_sb, in_=maskq_d.ap().rearrange("(q p) -> p q", p=P)
        )

        bo_sb = consts.tile([P, D], f32, tag="bo")
        nc.sync.dma_start(out=bo_sb, in_=bo_d.ap())

        ones_sb = consts.tile([P, 1], dt, tag="ones")
        nc.vector.memset(ones_sb, 1.0)

        # causal min-mask patterns for diagonal chunks: [P, GW] f32,
        # keep (BIG) where s_local >= t_local + off, else RAWNEG.
        mt = []
        for off in (0, 128):
            t_ = consts.tile([P, GW], f32, tag=f"mt{off}")
            nc.vector.memset(t_, BIG)
            nc.gpsimd.affine_select(
                out=t_, in_=t_,
                compare_op=mybir.AluOpType.is_ge,
                fill=RAWNEG, base=-off, channel_multiplier=-1,
                pattern=[[1, GW]],
            )
            mt.append(t_)

        # out accumulator, init = bo * maskq (bo has maskq-invariant fold done host-side)
        out_acc = consts.tile([P, NQT, D], f32, tag="out_acc")
        for qt in range(NQT):
            nc.vector.tensor_scalar_mul(
                out=out_acc[:, qt, :], in0=bo_sb, scalar1=maskq_sb[:, qt : qt + 1]
            )

        # ---- per-head pipeline ----
        n_heads = int(os.environ.get("MHA_HEADS", str(H)))
        for h in range(n_heads):
            m_sb = wpool.tile([P, DC, D], dt, tag="m")
            n_sb = wpool.tile([P, DC, D], dt, tag="n")
            nc.sync.dma_start(
                out=m_sb, in_=m_d.ap()[h].rearrange("(c p) e -> p c e", p=P)
            )
            nc.sync.dma_start(
                out=n_sb, in_=n_d.ap()[h].rearrange("(c p) e -> p c e", p=P)
            )

            # P1: Q'T [e, s]
            qp_sb = qpool.tile([P, DC, S], dt, tag="QT")
            for ec in range(DC):
                for sh in range(S // 512):
                    ps = ps_pj.tile([P, 512], f32, tag="pj")
                    for dc in range(DC):
                        nc.tensor.matmul(
                            ps,
                            m_sb[:, dc, ec * P : (ec + 1) * P],
                            xT_sb[:, dc, sh * 512 : (sh + 1) * 512],
                            start=(dc == 0),
                            stop=(dc == DC - 1),
                        )
                    nc.scalar.copy(
                        out=qp_sb[:, ec, sh * 512 : (sh + 1) * 512], in_=ps
                    )

            # P2: U [t, e]
            u_sb = upool.tile([P, NQT, D], dt, tag="U")
            for tt in range(NQT):
                psu = ps_pv.tile([P, D], f32, tag="pv", name="ps_u")
                for dc in range(DC):
                    nc.tensor.matmul(
                        psu,
                        xT_sb[:, dc, tt * P : (tt + 1) * P],
                        n_sb[:, dc, :],
                        start=(dc == 0),
                        stop=(dc == DC - 1),
                    )
                nc.vector.tensor_copy(out=u_sb[:, tt, :], in_=psu)

            # attention per 256-wide s-group; colsums accumulate per 512-half
            att_tiles = []
            ps_sums = None
            for qg in range(NG):
                ntt = 2 * qg + 2          # live t-chunks for this group
                s0 = qg * GW
                att_t = apool.tile([P, NQT, GW], dt, tag="attnT", name="att_t")
                att_tiles.append(att_t)
                if qg % 2 == 0:
                    ps_sums = ps_cs.tile([1, 512], f32, tag="cs")
                cso = (qg % 2) * GW
                for tt in range(ntt):
                    ps_s = ps_sc.tile([P, GW], f32, tag="sc")
                    for ec in range(DC):
                        nc.tensor.matmul(
                            ps_s,
                            xT_sb[:, ec, tt * P : (tt + 1) * P],
                            qp_sb[:, ec, s0 : s0 + GW],
                            start=(ec == 0),
                            stop=(ec == DC - 1),
                        )
                    if tt >= 2 * qg:  # diagonal chunk: causal min pre-exp
                        nc.vector.tensor_tensor(
                            out=ps_s, in0=ps_s, in1=mt[tt - 2 * qg],
                            op=mybir.AluOpType.min,
                        )
                    nc.scalar.activation(
                        out=att_t[:, tt, :],
                        in_=ps_s,
                        func=mybir.ActivationFunctionType.Exp,
                        scale=INV_SQRT_D,
                        bias=kb_sb[:, tt : tt + 1],
                    )
                for tt in range(ntt):
                    nc.tensor.matmul(
                        ps_sums[:, cso : cso + GW],
                        ones_sb,
                        att_t[:, tt, :],
                        start=(tt == 0),
                        stop=(tt == ntt - 1),
                    )

                if qg % 2 == 1:
                    # 512-half done: bounce colsums, recip, fold maskq
                    hh = qg // 2
                    srow = small.tile([1, 512], f32, tag="srow")
                    nc.vector.tensor_copy(out=srow, in_=ps_sums)
                    scr = scr_d.ap()[h * 2 + hh]
                    nc.sync.dma_start(out=scr, in_=srow)
                    scat = small.tile([P, 4], f32, tag="scat")
                    nc.sync.dma_start(
                        out=scat,
                        in_=bass.AP(
                            tensor=scr.tensor, offset=scr.offset, ap=[[1, P], [P, 4]]
                        ),
                    )
                    guard = small.tile([P, 4], f32, tag="guard")
                    nc.vector.tensor_scalar_add(out=guard, in0=scat, scalar1=1e-30)
                    recip = small.tile([P, 4], f32, tag="recip")
                    nc.vector.reciprocal(out=recip, in_=guard)
                    recipm = small.tile([P, 4], f32, tag="recipm")
                    nc.vector.tensor_tensor(
                        out=recipm, in0=recip,
                        in1=maskq_sb[:, hh * 4 : hh * 4 + 4],
                        op=mybir.AluOpType.mult,
                    )

                    # P4 for the half's 4 q-tiles (128-granular causality)
                    for qi in range(4):
                        qt = hh * 4 + qi
                        ps_p = ps_pv.tile([P, D], f32, tag="pv")
                        for tt in range(qt + 1):
                            # attn chunk [t, s-128] for this q-tile lives in
                            # group tile qt // 2, local col = (qt % 2)*128
                            nc.tensor.matmul(
                                ps_p,
                                att_tiles[qt // 2][
                                    :, tt, (qt % 2) * P : (qt % 2) * P + P
                                ],
                                u_sb[:, tt, :],
                                start=(tt == 0),
                                stop=(tt == qt),
                            )
                        nc.vector.scalar_tensor_tensor(
                            out=out_acc[:, qt, :],
                            in0=ps_p,
                            scalar=recipm[:, qi : qi + 1],
                            in1=out_acc[:, qt, :],
                            op0=mybir.AluOpType.mult,
                            op1=mybir.AluOpType.add,
                        )

        # ---- final store (maskq and bo already folded into out_acc) ----
        for qt in range(NQT):
            nc.sync.dma_start(
                out=out_d.ap()[qt * P : (qt + 1) * P, :], in_=out_acc[:, qt, :]
            )

    nc.compile()
    return nc


def _in_maps(x, mask, Wq, bq, Wk, bk, Wv, bv, Wo, bo, cfg):
    np_dt = _np_dt(cfg["dt"])
    f32 = np.float32
    x = np.asarray(x, f32)
    Wq = np.asarray(Wq, f32)
    Wk = np.asarray(Wk, f32)
    Wv = np.asarray(Wv, f32)
    Wo = np.asarray(Wo, f32).reshape(H, D, D)
    bq = np.asarray(bq, f32)
    bk = np.asarray(bk, f32)
    bv = np.asarray(bv, f32)
    bo = np.asarray(bo, f32)

    # host precompute: M = Wq Wk^T, N = Wv Wo  (fp32)
    M = np.einsum("hde,hfe->hdf", Wq, Wk)
    N = np.einsum("hde,hef->hdf", Wv, Wo)

    # bias folds (all-zero biases in this problem, kept for generality):
    #   scores += bq.K_t (per-key) -> raw bias columns; Q.bk const/row -> cancels
    #   out += sum_h (bv_h @ Wo_h) + bo  (attn rows sum to 1)
    bo_f = bo + np.einsum("hd,hdf->f", bv, Wo)

    m = np.asarray(mask) != 0
    maskq = m.astype(f32)

    shared = {
        "M": M.astype(np_dt),
        "N": N.astype(np_dt),
        "bo": np.broadcast_to(bo_f[None, :], (P, D)).copy(),
    }
    xT = np.ascontiguousarray(x.transpose(0, 2, 1))  # [B, D, S]
    maps = []
    for b in range(B):
        # per-key exp bias: 0 valid / KNEG masked; plus bq.K_t fold (zero here)
        kb = np.where(m[b], 0.0, np.float32(KNEG)).astype(f32)
        maps.append(
            {
                "xT": xT[b].astype(np_dt),
                "kbT": np.ascontiguousarray(kb.reshape(NQT, P).T),
                "maskq": maskq[b],
                **shared,
            }
        )
    return maps


def run(inputs, trace=False, cfg=None):
    """inputs: dict from setup_inputs(). Returns (out [B,S,D] f32, results)."""
    from concourse.bass_utils import run_bass_kernel_spmd

    global _BUILT
    cfg = dict(CFG if cfg is None else cfg)
    if _BUILT is None or _BUILT[1] != cfg:
        _BUILT = (build(cfg), cfg)
    nc = _BUILT[0]
    in_maps = _in_maps(**inputs, cfg=cfg)
    res = run_bass_kernel_spmd(
        nc, in_maps, core_ids=list(range(B)), trace=trace
    )
    out = np.stack([np.asarray(res.results[b]["out"], np.float32) for b in range(B)])
    return out, res


def kernel(**inputs):
    out, _ = run(inputs, trace=False)
    return out


# revision 11
# speedup vs baseline: 1.4142x; 1.0271x over previous
"""Multi-head self-attention TRN2 kernel (data-parallel over batch).

Problem: B=8, S=1024, D=384, H=8, per-head full D->D projections,
causal + key-padding mask, softmax, out_linear (H*D)->D, query-mask output.

Sharding: batch b -> NeuronCore b (8 cores, no collectives).

Algebraic restructure (host precompute, exact):
  M_h = Wq_h @ Wk_h^T   ->  scores_raw = x M_h x^T   (K-projection eliminated)
  N_h = Wv_h @ Wo_h     ->  out = sum_h softmax(scores) @ (x N_h)  (out-proj eliminated)
  bias folds: Q.bk term is constant per query row -> cancels in softmax;
  bq.K term -> per-key exp bias column; (attn@bv)Wo = bv@Wo (softmax sums to 1)
  -> folded into bo on host. All biases are zero in this problem anyway.

Per-core dataflow (one batch element), transpose-free, all bf16 matmuls:
  xT [D,S] bf16 resident in SBUF (used 3 ways: P1/P2 moving, P3 stationary).
  For each head h:
    P1: Q'T[e,s] = M-chunks @ xT          (PE, psum [128,512], -> bf16 SBUF)
    P2: U[t,e]   = xT-chunks @ N          (PE, psum [128,384], -> bf16 SBUF)
    per 256-wide q group (4 groups, causally-live t-chunks only):
      P3: scoresT[t,s] psum = xT-chunk stationary @ Q'T   (raw, unscaled)
      diagonal chunks: min(scores, MTpat) in-psum (DVE), 2 const patterns
      attnT[t,s] = exp(scores*inv_sqrt_d + keybias[t]) -> bf16 SBUF (ACT)
      colsum[1,s] += ones^T @ attnT       (PE, M=1 matmul, per 512-half)
    per 512-half: colsum -> DRAM bounce -> [128,4] -> +eps -> recip -> *maskq
    P4 per q-tile (128): psum[s,e] = sum_t attnT-chunk stationary @ U
      out_acc[s,:] += psum * recip'[s]    (DVE STT)
  out = out_acc (maskq,bo pre-folded) -> DRAM
"""

import os
from contextlib import ExitStack

import numpy as np

B, S, D, H = 8, 1024, 384, 8
P = 128
DC = D // P          # 3 partition chunks of the d/e axes
NQT = S // P         # 8 q/t tiles of 128
GW = 256             # q-group width for scores/exp
NG = S // GW         # 4 groups
BIG = 3.0e38
INV_SQRT_D = float(1.0 / np.sqrt(np.float32(D), dtype=np.float32))
KNEG = -120.0                      # exp bias for masked keys -> exp==0 in bf16
RAWNEG = float(KNEG / INV_SQRT_D)  # raw-score causal fill; scaled -> -120

CFG = {"dt": os.environ.get("MHA_DT", "bf16")}

_BUILT = None  # (nc, cfg)


def _dt(kind):
    import concourse.mybir as mybir

    return {"bf16": mybir.dt.bfloat16, "f32r": mybir.dt.float32r,
            "f32": mybir.dt.float32}[kind]


def _np_dt(kind):
    import ml_dtypes

    return ml_dtypes.bfloat16 if kind == "bf16" else np.float32


def build(cfg=None):
    import concourse.bass as bass
    import concourse.bacc as bacc
    import concourse.tile as tile
    import concourse.mybir as mybir

    cfg = dict(CFG if cfg is None else cfg)
    f32 = mybir.dt.float32
    u32 = mybir.dt.uint32
    dt = _dt(cfg["dt"])

    nc = bacc.Bacc("TRN2", target_bir_lowering=False, debug=False)

    xT_d = nc.dram_tensor("xT", [D, S], dt, kind="ExternalInput")
    m_d = nc.dram_tensor("M", [H, D, D], dt, kind="ExternalInput")
    n_d = nc.dram_tensor("N", [H, D, D], dt, kind="ExternalInput")
    kb_d = nc.dram_tensor("kbT", [P, NQT], f32, kind="ExternalInput")
    maskq_d = nc.dram_tensor("maskq", [S], f32, kind="ExternalInput")
    bo_d = nc.dram_tensor("bo", [P, D], f32, kind="ExternalInput")
    out_d = nc.dram_tensor("out", [S, D], f32, kind="ExternalOutput")
    # per-(head, s-half) bounce rows for column sums
    scr_d = nc.dram_tensor("sum_scratch", [H * 2, 512], f32)

    with tile.TileContext(nc) as tc, ExitStack() as ctx:
        consts = ctx.enter_context(tc.tile_pool(name="consts", bufs=1))
        wpool = ctx.enter_context(tc.tile_pool(name="wpool", bufs=2))
        qpool = ctx.enter_context(tc.tile_pool(name="qpool", bufs=2))
        upool = ctx.enter_context(tc.tile_pool(name="upool", bufs=2))
        apool = ctx.enter_context(tc.tile_pool(name="apool", bufs=3))
        small = ctx.enter_context(tc.tile_pool(name="small", bufs=8))
        ps_pj = ctx.enter_context(tc.tile_pool(name="ps_pj", bufs=2, space="PSUM"))
        ps_sc = ctx.enter_context(tc.tile_pool(name="ps_sc", bufs=2, space="PSUM"))
        ps_pv = ctx.enter_context(tc.tile_pool(name="ps_pv", bufs=3, space="PSUM"))
        ps_cs = ctx.enter_context(tc.tile_pool(name="ps_cs", bufs=1, space="PSUM"))

        # ---- PE warm-up under the initial DMA shadow (release clock gate)
        warm = consts.tile([P, 512], dt, tag="warm")
        nc.vector.memset(warm, 0)
        ps_w = ps_pj.tile([P, 512], f32, tag="pj", name="ps_warm")
        for _ in range(24):
            nc.tensor.matmul(ps_w, warm[:, :P], warm, start=True, stop=True)

        # ---- setup ----
        # xT as two s-half tiles so P1 can start after the first half lands
        xTh = []
        for sh in range(2):
            t_ = consts.tile([P, DC, 512], dt, tag=f"xT{sh}")
            nc.sync.dma_start(
                out=t_,
                in_=xT_d.ap()[:, sh * 512 : (sh + 1) * 512].rearrange(
                    "(c p) s -> p c s", p=P
                ),
            )
            xTh.append(t_)

        kb_sb = consts.tile([P, NQT], f32, tag="kbT")
        nc.sync.dma_start(out=kb_sb, in_=kb_d.ap())

        maskq_sb = consts.tile([P, NQT], f32, tag="maskq")
        nc.sync.dma_start(
            out=maskq_sb, in_=maskq_d.ap().rearrange("(q p) -> p q", p=P)
        )

        bo_sb = consts.tile([P, D], f32, tag="bo")
        nc.sync.dma_start(out=bo_sb, in_=bo_d.ap())

        ones_sb = consts.tile([P, 1], dt, tag="ones")
        nc.vector.memset(ones_sb, 1.0)

        # causal min-mask patterns for diagonal chunks: [P, GW] f32,
        # keep (BIG) where s_local >= t_local + off, else RAWNEG.
        mt = []
        for off in (0, 128):
            t_ = consts.tile([P, GW], f32, tag=f"mt{off}")
            nc.vector.memset(t_, BIG)
            nc.gpsimd.affine_select(
                out=t_, in_=t_,
                compare_op=mybir.AluOpType.is_ge,
                fill=RAWNEG, base=-off, channel_multiplier=-1,
                pattern=[[1, GW]],
            )
            mt.append(t_)

        # out accumulator, init = bo * maskq (bo has maskq-invariant fold done host-side)
        out_acc = consts.tile([P, NQT, D], f32, tag="out_acc")
        for qt in range(NQT):
            nc.vector.tensor_scalar_mul(
                out=out_acc[:, qt, :], in0=bo_sb, scalar1=maskq_sb[:, qt : qt + 1]
            )

        # ---- per-head pipeline ----
        n_heads = int(os.environ.get("MHA_HEADS", str(H)))
        for h in range(n_heads):
            m_sb = wpool.tile([P, DC, D], dt, tag="m")
            n_sb = wpool.tile([P, DC, D], dt, tag="n")
            nc.sync.dma_start(
                out=m_sb, in_=m_d.ap()[h].rearrange("(c p) e -> p c e", p=P)
            )
            nc.sync.dma_start(
                out=n_sb, in_=n_d.ap()[h].rearrange("(c p) e -> p c e", p=P)
            )

            # P1: Q'T [e, s]
            qp_sb = qpool.tile([P, DC, S], dt, tag="QT")
            for sh in range(S // 512):
                for ec in range(DC):
                    ps = ps_pj.tile([P, 512], f32, tag="pj")
                    for dc in range(DC):
                        nc.tensor.matmul(
                            ps,
                            m_sb[:, dc, ec * P : (ec + 1) * P],
                            xTh[sh][:, dc, :],
                            start=(dc == 0),
                            stop=(dc == DC - 1),
                        )
                    nc.scalar.copy(
                        out=qp_sb[:, ec, sh * 512 : (sh + 1) * 512], in_=ps
                    )

            # P2: U [t, e]
            u_sb = upool.tile([P, NQT, D], dt, tag="U")
            for tt in range(NQT):
                psu = ps_pv.tile([P, D], f32, tag="pv", name="ps_u")
                for dc in range(DC):
                    nc.tensor.matmul(
                        psu,
                        xTh[tt // 4][:, dc, (tt % 4) * P : (tt % 4 + 1) * P],
                        n_sb[:, dc, :],
                        start=(dc == 0),
                        stop=(dc == DC - 1),
                    )
                nc.vector.tensor_copy(out=u_sb[:, tt, :], in_=psu)

            # attention per 256-wide s-group; colsums accumulate per 512-half
            att_tiles = []
            ps_sums = None
            for qg in range(NG):
                ntt = 2 * qg + 2          # live t-chunks for this group
                s0 = qg * GW
                att_t = apool.tile([P, NQT, GW], dt, tag="attnT", name="att_t")
                att_tiles.append(att_t)
                if qg % 2 == 0:
                    ps_sums = ps_cs.tile([1, 512], f32, tag="cs")
                cso = (qg % 2) * GW
                for tt in range(ntt):
                    ps_s = ps_sc.tile([P, GW], f32, tag="sc")
                    for ec in range(DC):
                        nc.tensor.matmul(
                            ps_s,
                            xTh[tt // 4][:, ec, (tt % 4) * P : (tt % 4 + 1) * P],
                            qp_sb[:, ec, s0 : s0 + GW],
                            start=(ec == 0),
                            stop=(ec == DC - 1),
                        )
                    if tt >= 2 * qg:  # diagonal chunk: causal min pre-exp
                        nc.vector.tensor_tensor(
                            out=ps_s, in0=ps_s, in1=mt[tt - 2 * qg],
                            op=mybir.AluOpType.min,
                        )
                    nc.scalar.activation(
                        out=att_t[:, tt, :],
                        in_=ps_s,
                        func=mybir.ActivationFunctionType.Exp,
                        scale=INV_SQRT_D,
                        bias=kb_sb[:, tt : tt + 1],
                    )
                for tt in range(ntt):
                    nc.tensor.matmul(
                        ps_sums[:, cso : cso + GW],
                        ones_sb,
                        att_t[:, tt, :],
                        start=(tt == 0),
                        stop=(tt == ntt - 1),
                    )

                if qg % 2 == 1:
                    # 512-half done: bounce colsums, recip, fold maskq
                    hh = qg // 2
                    srow = small.tile([1, 512], f32, tag="srow")
                    nc.vector.tensor_copy(out=srow, in_=ps_sums)
                    scr = scr_d.ap()[h * 2 + hh]
                    nc.sync.dma_start(out=scr, in_=srow)
                    scat = small.tile([P, 4], f32, tag="scat")
                    nc.sync.dma_start(
                        out=scat,
                        in_=bass.AP(
                            tensor=scr.tensor, offset=scr.offset, ap=[[1, P], [P, 4]]
                        ),
                    )
                    guard = small.tile([P, 4], f32, tag="guard")
                    nc.vector.tensor_scalar_add(out=guard, in0=scat, scalar1=1e-30)
                    recip = small.tile([P, 4], f32, tag="recip")
                    nc.vector.reciprocal(out=recip, in_=guard)
                    recipm = small.tile([P, 4], f32, tag="recipm")
                    nc.vector.tensor_tensor(
                        out=recipm, in0=recip,
                        in1=maskq_sb[:, hh * 4 : hh * 4 + 4],
                        op=mybir.AluOpType.mult,
                    )

                    # P4 for the half's 4 q-tiles (128-granular causality)
                    for qi in range(4):
                        qt = hh * 4 + qi
                        ps_p = ps_pv.tile([P, D], f32, tag="pv")
                        for tt in range(qt + 1):
                            # attn chunk [t, s-128] for this q-tile lives in
                            # group tile qt // 2, local col = (qt % 2)*128
                            nc.tensor.matmul(
                                ps_p,
                                att_tiles[qt // 2][
                                    :, tt, (qt % 2) * P : (qt % 2) * P + P
                                ],
                                u_sb[:, tt, :],
                                start=(tt == 0),
                                stop=(tt == qt),
                            )
                        nc.vector.scalar_tensor_tensor(
                            out=out_acc[:, qt, :],
                            in0=ps_p,
                            scalar=recipm[:, qi : qi + 1],
                            in1=out_acc[:, qt, :],
                            op0=mybir.AluOpType.mult,
                            op1=mybir.AluOpType.add,
                        )

        # ---- final store (maskq and bo already folded into out_acc) ----
        for qt in range(NQT):
            nc.sync.dma_start(
                out=out_d.ap()[qt * P : (qt + 1) * P, :], in_=out_acc[:, qt, :]
            )

    nc.compile()
    return nc


def _in_maps(x, mask, Wq, bq, Wk, bk, Wv, bv, Wo, bo, cfg):
    np_dt = _np_dt(cfg["dt"])
    f32 = np.float32
    x = np.asarray(x, f32)
    Wq = np.asarray(Wq, f32)
    Wk = np.asarray(Wk, f32)
    Wv = np.asarray(Wv, f32)
    Wo = np.asarray(Wo, f32).reshape(H, D, D)
    bq = np.asarray(bq, f32)
    bk = np.asarray(bk, f32)
    bv = np.asarray(bv, f32)
    bo = np.asarray(bo, f32)

    # host precompute: M = Wq Wk^T, N = Wv Wo  (fp32)
    M = np.einsum("hde,hfe->hdf", Wq, Wk)
    N = np.einsum("hde,hef->hdf", Wv, Wo)

    # bias folds (all-zero biases in this problem, kept for generality):
    #   scores += bq.K_t (per-key) -> raw bias columns; Q.bk const/row -> cancels
    #   out += sum_h (bv_h @ Wo_h) + bo  (attn rows sum to 1)
    bo_f = bo + np.einsum("hd,hdf->f", bv, Wo)

    m = np.asarray(mask) != 0
    maskq = m.astype(f32)

    shared = {
        "M": M.astype(np_dt),
        "N": N.astype(np_dt),
        "bo": np.broadcast_to(bo_f[None, :], (P, D)).copy(),
    }
    xT = np.ascontiguousarray(x.transpose(0, 2, 1))  # [B, D, S]
    maps = []
    for b in range(B):
        # per-key exp bias: 0 valid / KNEG masked; plus bq.K_t fold (zero here)
        kb = np.where(m[b], 0.0, np.float32(KNEG)).astype(f32)
        maps.append(
            {
                "xT": xT[b].astype(np_dt),
                "kbT": np.ascontiguousarray(kb.reshape(NQT, P).T),
                "maskq": maskq[b],
                **shared,
            }
        )
    return maps


def run(inputs, trace=False, cfg=None):
    """inputs: dict from setup_inputs(). Returns (out [B,S,D] f32, results)."""
    from concourse.bass_utils import run_bass_kernel_spmd

    global _BUILT
    cfg = dict(CFG if cfg is None else cfg)
    if _BUILT is None or _BUILT[1] != cfg:
        _BUILT = (build(cfg), cfg)
    nc = _BUILT[0]
    in_maps = _in_maps(**inputs, cfg=cfg)
    res = run_bass_kernel_spmd(
        nc, in_maps, core_ids=list(range(B)), trace=trace
    )
    out = np.stack([np.asarray(res.results[b]["out"], np.float32) for b in range(B)])
    return out, res


def kernel(**inputs):
    out, _ = run(inputs, trace=False)
    return out
